# revision 18
# baseline (speedup 1.0000x reference)
"""Fused LayerNorm + 8-head attention + out-projection for Trainium2.

Problem: x[4, 2048, 512] -> LN -> QKV(512x1536) -> 8-head attention (S=2048,
Dh=64, materialized softmax) -> out-proj (512x512) + b_out.

Sharding: 8 cores = (batch, query-half). Each core gets the full batch-b
sequence (rotated so its 1024 query tokens are rows 0:1024 — attention over
keys is permutation invariant), computes k/v for all 2048 keys (redundant
with its pair core, but avoids any collective), and writes a disjoint
[1024, 512] slice of the output. No inter-core communication.

On-core dataflow (all matmuls bf16, f32 accumulation in PSUM):
  LN (bn_stats/bn_aggr, f32) -> xn bf16 -> PE-transpose -> xnT
  qT/kT = W_qk^T @ xnT   (gamma folded into W rows, beta via betaW row adds)
  v     = xnT^T @ W_v    (+ ones-row x betaW_v K=1 matmul)
  per head pair (row-tiled QK on the 128x128 PE array, heads at partition
  offsets 0/64):  scoresT[keys, qtok] -> exp(0.125*s) on ScalarE -> bf16
  AV: attn_T[dh, qtok] = v^T @ exp  with a concurrent col-tiled ones-matmul
  producing the softmax denominators replicated over 64 partitions in the
  same PSUM bank; reciprocal_approx_fast + SBUF->SBUF DMA partition move +
  tensor_tensor multiply normalize into attn_allT.
  out = attn_allT^T @ W_out + b_out (K=1 ones-row matmul), f32 out.
"""

import numpy as np

B, S, D = 4, 2048, 512
HEADS, DH = 8, 64
INNER = HEADS * DH  # 512
SQ = S // 2  # query tokens per core
SCALE = DH ** -0.5
LN_EPS = 1e-5
NT = S // 128  # 16 token tiles
NC_CORES = 8

_CACHED = {}


def _patch_tile_drain():
    """This container's walrus build rejects >1 sync wait on the Tile
    kernel-tail Drain ("Too many sync wait commands"). Spread the tail waits
    over extra SP nops, one per instruction."""
    import concourse.tile as tile_mod
    from concourse import mybir

    if getattr(tile_mod.TileContext, "_drain_patched", False):
        return

    def _drain_and_barrier(self, tick_clock, wait_clock):
        nc = self.nc
        drain_inst = nc.sync.drain()
        wait_clock.add_sem_waits(
            drain_inst.ins, tile_mod.ScopedClock({None: tick_clock.global_clock})
        )
        si = drain_inst.ins.sync_info
        if si is not None and si.on_wait and len(si.on_wait) > 1:
            waits = list(si.on_wait)
            drain_inst.ins.sync_info = mybir.SyncInfo(
                on_wait=waits[:1], on_update=list(si.on_update or [])
            )
            for i in range(1, len(waits)):
                nop = nc.sync.nop()
                nop.ins.sync_info = mybir.SyncInfo(
                    on_wait=waits[i : i + 1], on_update=[]
                )
        nc.all_engine_barrier()
        assert self.sems is not None
        popped = nc._tile_sem_poison_stack.pop()
        assert popped is self._sem_poison
        nc.clear_and_free_semaphores(list(self.sems.allocated().values()))
        nc.all_engine_barrier()

    tile_mod.TileContext._drain_and_barrier = _drain_and_barrier
    tile_mod.TileContext._drain_patched = True


def build_bass(split_waits=True):
    import concourse.bass as bass
    import concourse.tile as tile
    from concourse import mybir
    from concourse.masks import make_identity

    _patch_tile_drain()

    f32 = mybir.dt.float32
    bf16 = mybir.dt.bfloat16
    Alu = mybir.AluOpType
    Act = mybir.ActivationFunctionType

    nc = bass.Bass()
    x_d = nc.declare_dram_parameter("x", [S, D], f32, isOutput=False)
    wqkv_d = nc.declare_dram_parameter("w_qkv", [D, 3 * INNER], f32, isOutput=False)
    wout_d = nc.declare_dram_parameter("w_out", [INNER, D], f32, isOutput=False)
    gamma_d = nc.declare_dram_parameter("ln_gamma", [D], f32, isOutput=False)
    beta_d = nc.declare_dram_parameter("ln_beta", [D], f32, isOutput=False)
    bout_d = nc.declare_dram_parameter("b_out", [D], f32, isOutput=False)
    out_d = nc.declare_dram_parameter("out", [SQ, D], f32, isOutput=True)

    with tile.TileContext(nc) as tc:
        _build_body(nc, tc, tile, mybir, make_identity, Alu, Act, f32, bf16,
                    x_d, wqkv_d, wout_d, gamma_d, beta_d, bout_d, out_d)
    if split_waits:
        _split_excess_waits(nc, mybir)
    return nc


def _split_excess_waits(nc, mybir, max_waits=1):
    """This container's walrus build allows at most one sync wait per
    instruction. Hoist extra waits onto same-engine NoOps placed just before
    the instruction (engine streams are in-order, so semantics are
    preserved)."""
    import bass_rust

    k = 0
    for f in nc.m.functions:
        for blk in f.blocks:
            new_insts = []
            for ins in blk.instructions:
                si = ins.sync_info
                if si is not None and si.on_wait and len(si.on_wait) > max_waits:
                    waits = list(si.on_wait)
                    for i in range(max_waits, len(waits)):
                        nop = bass_rust.InstNoOp(
                            name=f"I-wsplit-{k}", ins=[], outs=[]
                        )
                        k += 1
                        nop.engine = ins.engine
                        nop.sync_info = mybir.SyncInfo(
                            on_wait=waits[i : i + 1], on_update=[]
                        )
                        new_insts.append(nop)
                    ins.sync_info = mybir.SyncInfo(
                        on_wait=waits[:max_waits],
                        on_update=list(si.on_update or []),
                    )
                new_insts.append(ins)
            if len(new_insts) != len(blk.instructions):
                blk.instructions = new_insts


def _build_body(nc, tc, tile, mybir, make_identity, Alu, Act, f32, bf16,
                x_d, wqkv_d, wout_d, gamma_d, beta_d, bout_d, out_d):
    from contextlib import ExitStack

    ctx = ExitStack()
    with ctx:
        consts = ctx.enter_context(tc.tile_pool(name="consts", bufs=1))
        # "big" pool: 16KB/partition slots shared by transient f32 weight
        # staging, xn, and the exp'd score tiles of the attention pipeline.
        big = ctx.enter_context(tc.tile_pool(name="big", bufs=4))
        xp = ctx.enter_context(tc.tile_pool(name="xp", bufs=3))
        mvp = ctx.enter_context(tc.tile_pool(name="mvp", bufs=4))
        persist = ctx.enter_context(tc.tile_pool(name="persist", bufs=1))
        recipp = ctx.enter_context(tc.tile_pool(name="recipp", bufs=4))
        recmvp = ctx.enter_context(tc.tile_pool(name="recmvp", bufs=4))
        attp = ctx.enter_context(tc.tile_pool(name="attp", bufs=8))
        outp = ctx.enter_context(tc.tile_pool(name="outp", bufs=3))
        # PSUM: 3 x [128,1024]f32 (2 banks each) + 2 x [128,512]f32 (1 bank)
        pp_big = ctx.enter_context(tc.tile_pool(name="pp_big", bufs=2, space="PSUM"))
        pp_av = ctx.enter_context(tc.tile_pool(name="pp_av", bufs=3, space="PSUM"))
        dramp = ctx.enter_context(tc.tile_pool(name="dramp", bufs=4, space="DRAM"))

        # ---- constants ----
        identity = consts.tile([128, 128], bf16)
        make_identity(nc, identity)
        ones64 = consts.tile([128, 64], bf16)
        nc.vector.memset(ones64, 1.0)
        ones_row = consts.tile([1, 128], bf16)
        nc.vector.memset(ones_row, 1.0)
        eps_t = consts.tile([128, 1], f32)
        nc.vector.memset(eps_t, LN_EPS)

        gammaT = consts.tile([128, 4], f32)
        nc.sync.dma_start(out=gammaT, in_=gamma_d.rearrange("(c p) -> p c", p=128))
        betaT_f = consts.tile([128, 4], f32)
        nc.sync.dma_start(out=betaT_f, in_=beta_d.rearrange("(c p) -> p c", p=128))
        betaT = consts.tile([128, 4], bf16)
        nc.vector.tensor_copy(out=betaT, in_=betaT_f)
        bout_f = consts.tile([1, D], f32)
        nc.sync.dma_start(out=bout_f, in_=bout_d[None, :])
        bout_row = consts.tile([1, D], bf16)
        nc.vector.tensor_copy(out=bout_row, in_=bout_f)

        # ---- weights: load f32, fold gamma into w_qkv rows, cast to bf16 ----
        wqkv_bf = persist.tile([128, 4, 3 * INNER], bf16, tag="wqkv_bf")
        for c in range(4):
            wf = big.tile([128, 3 * INNER], f32, tag="big")
            nc.sync.dma_start(out=wf, in_=wqkv_d[c * 128:(c + 1) * 128, :])
            nc.vector.tensor_scalar_mul(
                out=wqkv_bf[:, c, :], in0=wf, scalar1=gammaT[:, c:c + 1]
            )
        wout_f = big.tile([128, 4, D], f32, tag="big")
        nc.sync.dma_start(out=wout_f, in_=wout_d.rearrange("(c p) n -> p c n", p=128))
        wout_bf = persist.tile([128, 4, D], bf16, tag="wout_bf")
        nc.vector.tensor_copy(
            out=wout_bf.rearrange("p c n -> p (c n)"),
            in_=wout_f.rearrange("p c n -> p (c n)"),
        )

        # betaW_qk[m] = beta @ W_qk (per qk M-tile, per-partition scalars)
        betaWqk = consts.tile([128, 8], f32)
        for m in range(8):
            ps = pp_big.tile([128, 1], f32, tag="sc")
            for c in range(4):
                nc.tensor.matmul(
                    ps, lhsT=wqkv_bf[:, c, m * 128:(m + 1) * 128],
                    rhs=betaT[:, c:c + 1], start=(c == 0), stop=(c == 3),
                )
            nc.vector.tensor_copy(out=betaWqk[:, m:m + 1], in_=ps)
        # betaW_v = beta @ W_v (row [1, 512])
        betaWv = consts.tile([1, INNER], bf16)
        psv = pp_big.tile([1, INNER], f32, tag="sc")
        for c in range(4):
            nc.tensor.matmul(
                psv, lhsT=betaT[:, c:c + 1],
                rhs=wqkv_bf[:, c, 2 * INNER:3 * INNER],
                start=(c == 0), stop=(c == 3),
            )
        nc.vector.tensor_copy(out=betaWv, in_=psv)

        # ---- LayerNorm -> xn (bf16) ----
        xn = big.tile([128, NT, D], bf16, tag="big")
        for i in range(NT):
            xt = xp.tile([128, D], f32, tag="x")
            nc.sync.dma_start(out=xt, in_=x_d[i * 128:(i + 1) * 128, :])
            st = mvp.tile([128, 6], f32, tag="st")
            nc.vector.bn_stats(out=st, in_=xt)
            mv = mvp.tile([128, 2], f32, tag="mv")
            nc.vector.bn_aggr(out=mv, in_=st)
            nc.scalar.activation(out=mv[:, 1:2], in_=mv[:, 1:2], func=Act.Sqrt,
                                 bias=eps_t)
            nc.vector.reciprocal(out=mv[:, 1:2], in_=mv[:, 1:2])
            nc.vector.tensor_scalar(
                out=xn[:, i, :], in0=xt, scalar1=mv[:, 0:1], scalar2=mv[:, 1:2],
                op0=Alu.subtract, op1=Alu.mult,
            )

        # ---- transpose xn -> xnT[c] [128, 2048] ----
        xnT = [persist.tile([128, S], bf16, tag=f"xnT{c}", name=f"xnT{c}") for c in range(4)]
        for g in range(4):
            for c in range(4):
                pt = pp_big.tile([128, 512], bf16, tag="sc")
                for j2 in range(4):
                    nc.tensor.transpose(
                        pt[:, j2 * 128:(j2 + 1) * 128],
                        xn[:, g * 4 + j2, c * 128:(c + 1) * 128],
                        identity,
                    )
                nc.vector.tensor_copy(out=xnT[c][:, g * 512:(g + 1) * 512], in_=pt)

        # ---- projections ----
        qT = [persist.tile([128, SQ], bf16, tag=f"qT{m}", name=f"qT{m}") for m in range(4)]
        kT = [persist.tile([128, S], bf16, tag=f"kT{m}", name=f"kT{m}") for m in range(4)]
        for m in range(4):
            for n2 in range(2):  # q: tokens 0:1024
                ps = pp_big.tile([128, 512], f32, tag="sc")
                for c in range(4):
                    nc.tensor.matmul(
                        ps, lhsT=wqkv_bf[:, c, m * 128:(m + 1) * 128],
                        rhs=xnT[c][:, n2 * 512:(n2 + 1) * 512],
                        start=(c == 0), stop=(c == 3),
                    )
                nc.vector.tensor_scalar_add(
                    out=qT[m][:, n2 * 512:(n2 + 1) * 512], in0=ps,
                    scalar1=betaWqk[:, m:m + 1],
                )
            for n2 in range(4):  # k: all 2048 keys
                ps = pp_big.tile([128, 512], f32, tag="sc")
                for c in range(4):
                    nc.tensor.matmul(
                        ps, lhsT=wqkv_bf[:, c, INNER + m * 128:INNER + (m + 1) * 128],
                        rhs=xnT[c][:, n2 * 512:(n2 + 1) * 512],
                        start=(c == 0), stop=(c == 3),
                    )
                nc.vector.tensor_scalar_add(
                    out=kT[m][:, n2 * 512:(n2 + 1) * 512], in0=ps,
                    scalar1=betaWqk[:, 4 + m:5 + m],
                )
        # v with a ones column appended per even head: [64 v | 1 | 64 v] per
        # pair, so the even head's AV matmul (M=65) also produces the softmax
        # denominator row for free.
        v_sb = persist.tile([128, NT, 4, 129], bf16, tag="v_sb")
        nc.vector.memset(v_sb[:, :, :, 64:65], 1.0)

        def emit_vproj(t):
            ps = pp_big.tile([128, 512], f32, tag="sc", name="vps")
            for c in range(4):
                nc.tensor.matmul(
                    ps, lhsT=xnT[c][:, t * 128:(t + 1) * 128],
                    rhs=wqkv_bf[:, c, 2 * INNER:3 * INNER],
                    start=(c == 0), stop=False,
                )
            nc.tensor.matmul(ps, lhsT=ones_row, rhs=betaWv, start=False, stop=True)
            psv = ps.rearrange("p (j two d) -> p j two d", j=4, two=2)
            nc.vector.tensor_copy(out=v_sb[:, t, :, 0:64], in_=psv[:, :, 0, :])
            nc.vector.tensor_copy(out=v_sb[:, t, :, 65:129], in_=psv[:, :, 1, :])

        for t in range(NT):
            emit_vproj(t)

        # ---- attention ----
        import concourse.bass as bass_mod
        GROUPS = [(2 * g, 2 * g + 1) for g in range(8)]

        def emit_av(psA, psB, psS, expA, expB, grp, j):
            # strict PSUM-bank rotation psA -> psB -> psS per key tile: a
            # same-bank back-to-back matmul pair micro-stalls the PE queue
            # and oscillates HAM (measured +40% on every matmul).
            for kt in grp:
                st = kt == 0
                sp = kt == NT - 1
                nc.tensor.matmul(
                    psA[0:65, :], lhsT=v_sb[:, kt, j, 0:65],
                    rhs=expA[:, kt, :], start=st, stop=sp,
                )
                nc.tensor.matmul(
                    psB[64:128, :], lhsT=v_sb[:, kt, j, 65:129],
                    rhs=expB[:, kt, :], start=st, stop=sp,
                )
                nc.tensor.matmul(
                    psS[0:1, :], lhsT=ones64[:, 0:1], rhs=expB[:, kt, :],
                    start=st, stop=sp,
                )

        att_qc = {}
        for qc in range(2):
            att = [attp.tile([128, 512], bf16, tag="att", name="att") for _ in range(4)]
            att_qc[qc] = att
            for j in range(4):  # head pair (2j at partitions 0:64, 2j+1 at 64:128)
                expA = big.tile([128, NT, 512], bf16, tag="big")
                expB = big.tile([128, NT, 512], bf16, tag="big")
                psA = pp_av.tile([128, 512], f32, tag="av")
                psB = pp_av.tile([128, 512], f32, tag="av")
                psS = pp_av.tile([128, 512], f32, tag="av")
                for gi, grp in enumerate(GROUPS):
                    ssA = pp_big.tile([128, 1024], f32, tag="sc", name="ssA")
                    ssB = pp_big.tile([128, 1024], f32, tag="sc", name="ssB")
                    for kk, kt in enumerate(grp):
                        for base, ss in ((0, ssA), (64, ssB)):
                            nc.tensor.matmul(
                                ss[:, kk * 512:(kk + 1) * 512],
                                lhsT=kT[j][base:base + 64, kt * 128:(kt + 1) * 128],
                                rhs=qT[j][base:base + 64, qc * 512:(qc + 1) * 512],
                            )
                    g0 = grp[0]
                    for ss, expT in ((ssA, expA), (ssB, expB)):
                        nc.scalar.activation(
                            out=expT[:, g0:g0 + len(grp), :].rearrange(
                                "p a b -> p (a b)"),
                            in_=ss[:, 0:len(grp) * 512], func=Act.Exp,
                            scale=float(SCALE),
                        )
                    # AV + denominators for the PREVIOUS group: PE never waits
                    # on the exp it just enabled.
                    if gi > 0:
                        emit_av(psA, psB, psS, expA, expB, GROUPS[gi - 1], j)
                emit_av(psA, psB, psS, expA, expB, GROUPS[-1], j)
                # Drain unnormalized attention + denominator rows to SBUF
                # immediately so the PSUM banks recycle without waiting on the
                # normalization DMA chain.
                nc.vector.tensor_copy(out=att[j][0:64, :], in_=psA[0:64, :])
                nc.vector.tensor_copy(out=att[j][64:128, :], in_=psB[64:128, :])
                srow = recipp.tile([128, 512], f32, tag="srow")
                nc.vector.tensor_copy(out=srow[64:65, :], in_=psA[64:65, :])
                nc.vector.tensor_copy(out=srow[0:1, :], in_=psS[0:1, :])
                # Gather the 512 denominators onto 128 partitions, one cheap
                # reciprocal, scatter to DRAM, stride-0-broadcast back, and
                # multiply in place.
                recT = recipp.tile([128, 8], f32, tag="recT")
                nc.sync.dma_start(out=recT[:, 0:4], in_=srow[64:65, :])
                nc.sync.dma_start(out=recT[:, 4:8], in_=srow[0:1, :])
                nc.vector.reciprocal(out=recT, in_=recT)
                recTb = recipp.tile([128, 8], bf16, tag="recTb")
                nc.vector.tensor_copy(out=recTb, in_=recT)
                dsA = dramp.tile([512], bf16, tag="dsA", name="dsA")
                dsB = dramp.tile([512], bf16, tag="dsB", name="dsB")
                nc.sync.dma_start(out=dsA, in_=recTb[:, 0:4])
                nc.sync.dma_start(out=dsB, in_=recTb[:, 4:8])
                rb = recmvp.tile([128, 512], bf16, tag="rb")
                bcastA = bass_mod.AP(tensor=dsA.tensor, offset=dsA.offset,
                                     ap=[[0, 64]] + [list(a) for a in dsA.ap])
                bcastB = bass_mod.AP(tensor=dsB.tensor, offset=dsB.offset,
                                     ap=[[0, 64]] + [list(a) for a in dsB.ap])
                nc.sync.dma_start(out=rb[0:64, :], in_=bcastA)
                nc.sync.dma_start(out=rb[64:128, :], in_=bcastB)
                nc.vector.tensor_mul(out=att[j][0:64, :], in0=att[j][0:64, :],
                                     in1=rb[0:64, :])
                nc.vector.tensor_mul(out=att[j][64:128, :],
                                     in0=att[j][64:128, :], in1=rb[64:128, :])
        # ---- out-projections (after ALL attention units, so the PE fills
        # the normalization-chain latency with the other chunk's work) ----
        for qc in range(2):
            att = att_qc[qc]
            for t in range(4):
                po = pp_big.tile([128, 512], f32, tag="sc")
                for c in range(4):
                    nc.tensor.matmul(
                        po, lhsT=att[c][:, t * 128:(t + 1) * 128],
                        rhs=wout_bf[:, c, :], start=(c == 0), stop=False,
                    )
                nc.tensor.matmul(po, lhsT=ones_row, rhs=bout_row,
                                 start=False, stop=True)
                ot = outp.tile([128, 512], f32, tag="ot")
                nc.vector.tensor_copy(out=ot, in_=po)
                row0 = qc * 512 + t * 128
                nc.sync.dma_start(out=out_d[row0:row0 + 128, :], in_=ot)


def _get_nc():
    if "nc" not in _CACHED:
        _CACHED["nc"] = build_bass()
    return _CACHED["nc"]


def shard_inputs(x, w_qkv, w_out, ln_gamma, ln_beta, b_out):
    in_maps = []
    for c in range(NC_CORES):
        b, half = c // 2, c % 2
        xb = x[b]
        if half:
            xb = np.concatenate([xb[SQ:], xb[:SQ]], axis=0)
        in_maps.append({
            "x": np.ascontiguousarray(xb, dtype=np.float32),
            "w_qkv": np.ascontiguousarray(w_qkv, dtype=np.float32),
            "w_out": np.ascontiguousarray(w_out, dtype=np.float32),
            "ln_gamma": np.ascontiguousarray(ln_gamma, dtype=np.float32),
            "ln_beta": np.ascontiguousarray(ln_beta, dtype=np.float32),
            "b_out": np.ascontiguousarray(b_out, dtype=np.float32),
        })
    return in_maps


def unshard_outputs(results):
    out = np.empty((B, S, D), dtype=np.float32)
    for c in range(NC_CORES):
        b, half = c // 2, c % 2
        out[b, half * SQ:(half + 1) * SQ] = results[c]["out"]
    return out


def kernel(x, ln_gamma, ln_beta, w_qkv, w_out, b_out, _trace=False):
    from concourse.bass_utils import run_bass_kernel_spmd

    x = np.asarray(x, dtype=np.float32)
    nc = _get_nc()
    in_maps = shard_inputs(x, np.asarray(w_qkv), np.asarray(w_out),
                           np.asarray(ln_gamma), np.asarray(ln_beta),
                           np.asarray(b_out))
    res = run_bass_kernel_spmd(nc, in_maps, core_ids=list(range(NC_CORES)),
                               trace=_trace)
    out = unshard_outputs(res.results)
    if _trace:
        return out, res
    return out


# revision 19
# speedup vs baseline: 1.3032x; 1.3032x over previous
"""Fused LayerNorm + 8-head attention + out-projection for Trainium2.

Problem: x[4, 2048, 512] -> LN -> QKV(512x1536) -> 8-head attention (S=2048,
Dh=64, materialized softmax) -> out-proj (512x512) + b_out.

Sharding: 8 cores = (batch, query-half). Each core gets the full batch-b
sequence (rotated so its 1024 query tokens are rows 0:1024 — attention over
keys is permutation invariant), computes k/v for all 2048 keys (redundant
with its pair core, but avoids any collective), and writes a disjoint
[1024, 512] slice of the output. No inter-core communication.

On-core dataflow (all matmuls bf16, f32 accumulation in PSUM):
  LN (bn_stats/bn_aggr, f32) -> xn bf16 -> PE-transpose -> xnT
  qT/kT = W_qk^T @ xnT   (gamma folded into W rows, beta via betaW row adds)
  v     = xnT^T @ W_v    (+ ones-row x betaW_v K=1 matmul)
  per head pair (row-tiled QK on the 128x128 PE array, heads at partition
  offsets 0/64):  scoresT[keys, qtok] -> exp(0.125*s) on ScalarE -> bf16
  AV: attn_T[dh, qtok] = v^T @ exp  with a concurrent col-tiled ones-matmul
  producing the softmax denominators replicated over 64 partitions in the
  same PSUM bank; reciprocal_approx_fast + SBUF->SBUF DMA partition move +
  tensor_tensor multiply normalize into attn_allT.
  out = attn_allT^T @ W_out + b_out (K=1 ones-row matmul), f32 out.
"""

import numpy as np

B, S, D = 4, 2048, 512
HEADS, DH = 8, 64
INNER = HEADS * DH  # 512
SQ = S // 2  # query tokens per core
SCALE = DH ** -0.5
LN_EPS = 1e-5
NT = S // 128  # 16 token tiles
NC_CORES = 8

_CACHED = {}


def _patch_tile_drain():
    """This container's walrus build rejects >1 sync wait on the Tile
    kernel-tail Drain ("Too many sync wait commands"). Spread the tail waits
    over extra SP nops, one per instruction."""
    import concourse.tile as tile_mod
    from concourse import mybir

    if getattr(tile_mod.TileContext, "_drain_patched", False):
        return

    def _drain_and_barrier(self, tick_clock, wait_clock):
        nc = self.nc
        drain_inst = nc.sync.drain()
        wait_clock.add_sem_waits(
            drain_inst.ins, tile_mod.ScopedClock({None: tick_clock.global_clock})
        )
        si = drain_inst.ins.sync_info
        if si is not None and si.on_wait and len(si.on_wait) > 1:
            waits = list(si.on_wait)
            drain_inst.ins.sync_info = mybir.SyncInfo(
                on_wait=waits[:1], on_update=list(si.on_update or [])
            )
            for i in range(1, len(waits)):
                nop = nc.sync.nop()
                nop.ins.sync_info = mybir.SyncInfo(
                    on_wait=waits[i : i + 1], on_update=[]
                )
        nc.all_engine_barrier()
        assert self.sems is not None
        popped = nc._tile_sem_poison_stack.pop()
        assert popped is self._sem_poison
        nc.clear_and_free_semaphores(list(self.sems.allocated().values()))
        nc.all_engine_barrier()

    tile_mod.TileContext._drain_and_barrier = _drain_and_barrier
    tile_mod.TileContext._drain_patched = True


def build_bass(split_waits=True):
    import concourse.bass as bass
    import concourse.tile as tile
    from concourse import mybir
    from concourse.masks import make_identity

    _patch_tile_drain()

    f32 = mybir.dt.float32
    bf16 = mybir.dt.bfloat16
    Alu = mybir.AluOpType
    Act = mybir.ActivationFunctionType

    nc = bass.Bass()
    x_d = nc.declare_dram_parameter("x", [S, D], f32, isOutput=False)
    wqkv_d = nc.declare_dram_parameter("w_qkv", [D, 3 * INNER], f32, isOutput=False)
    wout_d = nc.declare_dram_parameter("w_out", [INNER, D], f32, isOutput=False)
    gamma_d = nc.declare_dram_parameter("ln_gamma", [D], f32, isOutput=False)
    beta_d = nc.declare_dram_parameter("ln_beta", [D], f32, isOutput=False)
    bout_d = nc.declare_dram_parameter("b_out", [D], f32, isOutput=False)
    out_d = nc.declare_dram_parameter("out", [SQ, D], f32, isOutput=True)

    with tile.TileContext(nc) as tc:
        _build_body(nc, tc, tile, mybir, make_identity, Alu, Act, f32, bf16,
                    x_d, wqkv_d, wout_d, gamma_d, beta_d, bout_d, out_d)
    if split_waits:
        _split_excess_waits(nc, mybir)
    return nc


def _split_excess_waits(nc, mybir, max_waits=1):
    """This container's walrus build allows at most one sync wait per
    instruction. Hoist extra waits onto same-engine NoOps placed just before
    the instruction (engine streams are in-order, so semantics are
    preserved)."""
    import bass_rust

    k = 0
    for f in nc.m.functions:
        for blk in f.blocks:
            new_insts = []
            for ins in blk.instructions:
                si = ins.sync_info
                if si is not None and si.on_wait and len(si.on_wait) > max_waits:
                    waits = list(si.on_wait)
                    for i in range(max_waits, len(waits)):
                        nop = bass_rust.InstNoOp(
                            name=f"I-wsplit-{k}", ins=[], outs=[]
                        )
                        k += 1
                        nop.engine = ins.engine
                        nop.sync_info = mybir.SyncInfo(
                            on_wait=waits[i : i + 1], on_update=[]
                        )
                        new_insts.append(nop)
                    ins.sync_info = mybir.SyncInfo(
                        on_wait=waits[:max_waits],
                        on_update=list(si.on_update or []),
                    )
                new_insts.append(ins)
            if len(new_insts) != len(blk.instructions):
                blk.instructions = new_insts


def _build_body(nc, tc, tile, mybir, make_identity, Alu, Act, f32, bf16,
                x_d, wqkv_d, wout_d, gamma_d, beta_d, bout_d, out_d):
    from contextlib import ExitStack

    ctx = ExitStack()
    with ctx:
        consts = ctx.enter_context(tc.tile_pool(name="consts", bufs=1))
        # "big" pool: 16KB/partition slots shared by transient f32 weight
        # staging, xn, and the exp'd score tiles of the attention pipeline.
        big = ctx.enter_context(tc.tile_pool(name="big", bufs=4))
        xp = ctx.enter_context(tc.tile_pool(name="xp", bufs=3))
        mvp = ctx.enter_context(tc.tile_pool(name="mvp", bufs=4))
        persist = ctx.enter_context(tc.tile_pool(name="persist", bufs=1))
        recipp = ctx.enter_context(tc.tile_pool(name="recipp", bufs=4))
        recmvp = ctx.enter_context(tc.tile_pool(name="recmvp", bufs=4))
        attp = ctx.enter_context(tc.tile_pool(name="attp", bufs=8))
        outp = ctx.enter_context(tc.tile_pool(name="outp", bufs=3))
        # PSUM: 3 x [128,1024]f32 (2 banks each) + 2 x [128,512]f32 (1 bank)
        pp_big = ctx.enter_context(tc.tile_pool(name="pp_big", bufs=3, space="PSUM"))
        pp_av = ctx.enter_context(tc.tile_pool(name="pp_av", bufs=2, space="PSUM"))
        dramp = ctx.enter_context(tc.tile_pool(name="dramp", bufs=4, space="DRAM"))

        # ---- constants ----
        identity = consts.tile([128, 128], bf16)
        make_identity(nc, identity)
        ones64 = consts.tile([128, 64], bf16)
        nc.vector.memset(ones64, 1.0)
        ones_row = consts.tile([1, 128], bf16)
        nc.vector.memset(ones_row, 1.0)
        eps_t = consts.tile([128, 1], f32)
        nc.vector.memset(eps_t, LN_EPS)

        gammaT = consts.tile([128, 4], f32)
        nc.sync.dma_start(out=gammaT, in_=gamma_d.rearrange("(c p) -> p c", p=128))
        betaT_f = consts.tile([128, 4], f32)
        nc.sync.dma_start(out=betaT_f, in_=beta_d.rearrange("(c p) -> p c", p=128))
        betaT = consts.tile([128, 4], bf16)
        nc.vector.tensor_copy(out=betaT, in_=betaT_f)
        bout_f = consts.tile([1, D], f32)
        nc.sync.dma_start(out=bout_f, in_=bout_d[None, :])
        bout_row = consts.tile([1, D], bf16)
        nc.vector.tensor_copy(out=bout_row, in_=bout_f)

        # ---- weights: load f32, fold gamma into w_qkv rows, cast to bf16 ----
        wqkv_bf = persist.tile([128, 4, 3 * INNER], bf16, tag="wqkv_bf")
        for c in range(4):
            wf = big.tile([128, 3 * INNER], f32, tag="big")
            nc.sync.dma_start(out=wf, in_=wqkv_d[c * 128:(c + 1) * 128, :])
            nc.vector.tensor_scalar_mul(
                out=wqkv_bf[:, c, :], in0=wf, scalar1=gammaT[:, c:c + 1]
            )
        wout_f = big.tile([128, 4, D], f32, tag="big")
        nc.sync.dma_start(out=wout_f, in_=wout_d.rearrange("(c p) n -> p c n", p=128))
        wout_bf = persist.tile([128, 4, D], bf16, tag="wout_bf")
        nc.vector.tensor_copy(
            out=wout_bf.rearrange("p c n -> p (c n)"),
            in_=wout_f.rearrange("p c n -> p (c n)"),
        )

        # betaW_qk[m] = beta @ W_qk (per qk M-tile, per-partition scalars)
        betaWqk = consts.tile([128, 8], f32)
        for m in range(8):
            ps = pp_big.tile([128, 1], f32, tag="sc")
            for c in range(4):
                nc.tensor.matmul(
                    ps, lhsT=wqkv_bf[:, c, m * 128:(m + 1) * 128],
                    rhs=betaT[:, c:c + 1], start=(c == 0), stop=(c == 3),
                )
            nc.vector.tensor_copy(out=betaWqk[:, m:m + 1], in_=ps)
        # betaW_v = beta @ W_v (row [1, 512])
        betaWv = consts.tile([1, INNER], bf16)
        psv = pp_big.tile([1, INNER], f32, tag="sc")
        for c in range(4):
            nc.tensor.matmul(
                psv, lhsT=betaT[:, c:c + 1],
                rhs=wqkv_bf[:, c, 2 * INNER:3 * INNER],
                start=(c == 0), stop=(c == 3),
            )
        nc.vector.tensor_copy(out=betaWv, in_=psv)

        # ---- LayerNorm -> xn (bf16) ----
        xn = big.tile([128, NT, D], bf16, tag="big")
        for i in range(NT):
            xt = xp.tile([128, D], f32, tag="x")
            nc.sync.dma_start(out=xt, in_=x_d[i * 128:(i + 1) * 128, :])
            st = mvp.tile([128, 6], f32, tag="st")
            nc.vector.bn_stats(out=st, in_=xt)
            mv = mvp.tile([128, 2], f32, tag="mv")
            nc.vector.bn_aggr(out=mv, in_=st)
            nc.scalar.activation(out=mv[:, 1:2], in_=mv[:, 1:2], func=Act.Sqrt,
                                 bias=eps_t)
            nc.vector.reciprocal(out=mv[:, 1:2], in_=mv[:, 1:2])
            nc.vector.tensor_scalar(
                out=xn[:, i, :], in0=xt, scalar1=mv[:, 0:1], scalar2=mv[:, 1:2],
                op0=Alu.subtract, op1=Alu.mult,
            )

        # ---- transpose xn -> xnT[c] [128, 2048] ----
        xnT = [persist.tile([128, S], bf16, tag=f"xnT{c}", name=f"xnT{c}") for c in range(4)]
        for g in range(4):
            for c in range(4):
                pt = pp_big.tile([128, 512], bf16, tag="sc")
                for j2 in range(4):
                    nc.tensor.transpose(
                        pt[:, j2 * 128:(j2 + 1) * 128],
                        xn[:, g * 4 + j2, c * 128:(c + 1) * 128],
                        identity,
                    )
                nc.vector.tensor_copy(out=xnT[c][:, g * 512:(g + 1) * 512], in_=pt)

        # ---- projections ----
        qT = [persist.tile([128, SQ], bf16, tag=f"qT{m}", name=f"qT{m}") for m in range(4)]
        kT = [persist.tile([128, S], bf16, tag=f"kT{m}", name=f"kT{m}") for m in range(4)]
        for m in range(4):
            for n2 in range(2):  # q: tokens 0:1024
                ps = pp_big.tile([128, 512], f32, tag="sc")
                for c in range(4):
                    nc.tensor.matmul(
                        ps, lhsT=wqkv_bf[:, c, m * 128:(m + 1) * 128],
                        rhs=xnT[c][:, n2 * 512:(n2 + 1) * 512],
                        start=(c == 0), stop=(c == 3),
                    )
                nc.vector.tensor_scalar_add(
                    out=qT[m][:, n2 * 512:(n2 + 1) * 512], in0=ps,
                    scalar1=betaWqk[:, m:m + 1],
                )
            for n2 in range(4):  # k: all 2048 keys
                ps = pp_big.tile([128, 512], f32, tag="sc")
                for c in range(4):
                    nc.tensor.matmul(
                        ps, lhsT=wqkv_bf[:, c, INNER + m * 128:INNER + (m + 1) * 128],
                        rhs=xnT[c][:, n2 * 512:(n2 + 1) * 512],
                        start=(c == 0), stop=(c == 3),
                    )
                nc.vector.tensor_scalar_add(
                    out=kT[m][:, n2 * 512:(n2 + 1) * 512], in0=ps,
                    scalar1=betaWqk[:, 4 + m:5 + m],
                )
        # v with a ones column appended per even head: [64 v | 1 | 64 v] per
        # pair, so the even head's AV matmul (M=65) also produces the softmax
        # denominator row for free.
        v_sb = persist.tile([128, NT, 4, 130], bf16, tag="v_sb")
        nc.vector.memset(v_sb[:, :, :, 64:65], 1.0)
        nc.vector.memset(v_sb[:, :, :, 129:130], 1.0)

        def emit_vproj(t):
            ps = pp_big.tile([128, 512], f32, tag="sc", name="vps")
            for c in range(4):
                nc.tensor.matmul(
                    ps, lhsT=xnT[c][:, t * 128:(t + 1) * 128],
                    rhs=wqkv_bf[:, c, 2 * INNER:3 * INNER],
                    start=(c == 0), stop=False,
                )
            nc.tensor.matmul(ps, lhsT=ones_row, rhs=betaWv, start=False, stop=True)
            psv = ps.rearrange("p (j two d) -> p j two d", j=4, two=2)
            nc.vector.tensor_copy(out=v_sb[:, t, :, 0:64], in_=psv[:, :, 0, :])
            nc.vector.tensor_copy(out=v_sb[:, t, :, 65:129], in_=psv[:, :, 1, :])

        for t in range(NT):
            emit_vproj(t)

        # ---- attention ----
        import concourse.bass as bass_mod
        GROUPS = [(2 * g, 2 * g + 1) for g in range(8)]

        def emit_av(psA, psB, expA, expB, grp, j):
            # Both heads' AV fused with their denominator row (M=65, ones col
            # appended in v), alternating PSUM banks: back-to-back same-bank
            # matmuls micro-stall the PE queue and de-warm HAM.
            for kt in grp:
                st = kt == 0
                sp = kt == NT - 1
                nc.tensor.matmul(
                    psA[0:65, :], lhsT=v_sb[:, kt, j, 0:65],
                    rhs=expA[:, kt, :], start=st, stop=sp,
                )
                nc.tensor.matmul(
                    psB[0:65, :], lhsT=v_sb[:, kt, j, 65:130],
                    rhs=expB[:, kt, :], start=st, stop=sp,
                )

        att_qc = {}
        for qc in range(2):
            att = [attp.tile([128, 512], bf16, tag="att", name="att") for _ in range(4)]
            att_qc[qc] = att
            for j in range(4):  # head pair (2j at partitions 0:64, 2j+1 at 64:128)
                expA = big.tile([128, NT, 512], bf16, tag="big")
                expB = big.tile([128, NT, 512], bf16, tag="big")
                psA = pp_av.tile([128, 512], f32, tag="av")
                psB = pp_av.tile([128, 512], f32, tag="av")
                for gi, grp in enumerate(GROUPS):
                    for base, expT in ((0, expA), (64, expB)):
                        ss = pp_big.tile([128, 1024], f32, tag="sc")
                        for kk, kt in enumerate(grp):
                            nc.tensor.matmul(
                                ss[:, kk * 512:(kk + 1) * 512],
                                lhsT=kT[j][base:base + 64, kt * 128:(kt + 1) * 128],
                                rhs=qT[j][base:base + 64, qc * 512:(qc + 1) * 512],
                            )
                        g0 = grp[0]
                        nc.scalar.activation(
                            out=expT[:, g0:g0 + len(grp), :].rearrange(
                                "p a b -> p (a b)"),
                            in_=ss[:, 0:len(grp) * 512], func=Act.Exp,
                            scale=float(SCALE),
                        )
                    # AV for the PREVIOUS group: PE never waits on the exp it
                    # just enabled.
                    if gi > 0:
                        emit_av(psA, psB, expA, expB, GROUPS[gi - 1], j)
                emit_av(psA, psB, expA, expB, GROUPS[-1], j)
                # Drain unnormalized attention + denominator rows to SBUF
                # immediately so the PSUM banks recycle without waiting on the
                # normalization DMA chain. Head B's attention lands at
                # partitions 0:64 (its M=65 fused matmul) and is routed to
                # att[j][64:128] by a cross-partition SBUF->SBUF DMA.
                nc.vector.tensor_copy(out=att[j][0:64, :], in_=psA[0:64, :])
                battn = recipp.tile([128, 512], bf16, tag="battn")
                nc.vector.tensor_copy(out=battn[0:64, :], in_=psB[0:64, :])
                nc.sync.dma_start(out=att[j][64:128, :], in_=battn[0:64, :])
                srow = recipp.tile([128, 512], f32, tag="srow")
                srowB = recipp.tile([128, 512], f32, tag="srowB")
                nc.vector.tensor_copy(out=srow[64:65, :], in_=psA[64:65, :])
                nc.vector.tensor_copy(out=srowB[64:65, :], in_=psB[64:65, :])
                # Gather the 512 denominators onto 128 partitions, one cheap
                # reciprocal, scatter to DRAM, stride-0-broadcast back, and
                # multiply in place.
                recT = recipp.tile([128, 8], f32, tag="recT")
                nc.sync.dma_start(out=recT[:, 0:4], in_=srow[64:65, :])
                nc.sync.dma_start(out=recT[:, 4:8], in_=srowB[64:65, :])
                nc.vector.reciprocal(out=recT, in_=recT)
                recTb = recipp.tile([128, 8], bf16, tag="recTb")
                nc.vector.tensor_copy(out=recTb, in_=recT)
                dsA = dramp.tile([512], bf16, tag="dsA", name="dsA")
                dsB = dramp.tile([512], bf16, tag="dsB", name="dsB")
                nc.sync.dma_start(out=dsA, in_=recTb[:, 0:4])
                nc.sync.dma_start(out=dsB, in_=recTb[:, 4:8])
                rb = recmvp.tile([128, 512], bf16, tag="rb")
                bcastA = bass_mod.AP(tensor=dsA.tensor, offset=dsA.offset,
                                     ap=[[0, 64]] + [list(a) for a in dsA.ap])
                bcastB = bass_mod.AP(tensor=dsB.tensor, offset=dsB.offset,
                                     ap=[[0, 64]] + [list(a) for a in dsB.ap])
                nc.sync.dma_start(out=rb[0:64, :], in_=bcastA)
                nc.sync.dma_start(out=rb[64:128, :], in_=bcastB)
                nc.vector.tensor_mul(out=att[j][0:64, :], in0=att[j][0:64, :],
                                     in1=rb[0:64, :])
                nc.vector.tensor_mul(out=att[j][64:128, :],
                                     in0=att[j][64:128, :], in1=rb[64:128, :])
        # ---- out-projections (after ALL attention units, so the PE fills
        # the normalization-chain latency with the other chunk's work) ----
        for qc in range(2):
            att = att_qc[qc]
            for t in range(4):
                po = pp_big.tile([128, 512], f32, tag="sc")
                for c in range(4):
                    nc.tensor.matmul(
                        po, lhsT=att[c][:, t * 128:(t + 1) * 128],
                        rhs=wout_bf[:, c, :], start=(c == 0), stop=False,
                    )
                nc.tensor.matmul(po, lhsT=ones_row, rhs=bout_row,
                                 start=False, stop=True)
                ot = outp.tile([128, 512], f32, tag="ot")
                nc.vector.tensor_copy(out=ot, in_=po)
                row0 = qc * 512 + t * 128
                nc.sync.dma_start(out=out_d[row0:row0 + 128, :], in_=ot)


def _get_nc():
    if "nc" not in _CACHED:
        _CACHED["nc"] = build_bass()
    return _CACHED["nc"]


def shard_inputs(x, w_qkv, w_out, ln_gamma, ln_beta, b_out):
    in_maps = []
    for c in range(NC_CORES):
        b, half = c // 2, c % 2
        xb = x[b]
        if half:
            xb = np.concatenate([xb[SQ:], xb[:SQ]], axis=0)
        in_maps.append({
            "x": np.ascontiguousarray(xb, dtype=np.float32),
            "w_qkv": np.ascontiguousarray(w_qkv, dtype=np.float32),
            "w_out": np.ascontiguousarray(w_out, dtype=np.float32),
            "ln_gamma": np.ascontiguousarray(ln_gamma, dtype=np.float32),
            "ln_beta": np.ascontiguousarray(ln_beta, dtype=np.float32),
            "b_out": np.ascontiguousarray(b_out, dtype=np.float32),
        })
    return in_maps


def unshard_outputs(results):
    out = np.empty((B, S, D), dtype=np.float32)
    for c in range(NC_CORES):
        b, half = c // 2, c % 2
        out[b, half * SQ:(half + 1) * SQ] = results[c]["out"]
    return out


def kernel(x, ln_gamma, ln_beta, w_qkv, w_out, b_out, _trace=False):
    from concourse.bass_utils import run_bass_kernel_spmd

    x = np.asarray(x, dtype=np.float32)
    nc = _get_nc()
    in_maps = shard_inputs(x, np.asarray(w_qkv), np.asarray(w_out),
                           np.asarray(ln_gamma), np.asarray(ln_beta),
                           np.asarray(b_out))
    res = run_bass_kernel_spmd(nc, in_maps, core_ids=list(range(NC_CORES)),
                               trace=_trace)
    out = unshard_outputs(res.results)
    if _trace:
        return out, res
    return out
